# revision 9
# baseline (speedup 1.0000x reference)
"""Trainium2 Bass kernel for nn_ClusterMemory (scatter_memory).

Strategy (8 NeuronCores, SPMD, no collectives):
  - The three feature banks [32768, 2048] are sharded row-wise, 4096 rows per
    core.  Each core streams its bank shards (pre-transposed to [K, S] layout,
    bf16) from HBM exactly once and computes its logit shard
    logits[b, s] = (x_norm[b]/TEMP) . f[s] with fp32 PSUM accumulation.
  - Softmax-CE: logits are bounded (|l| < 3 for unit-norm inputs), so each
    core computes per-row sum(exp(logits_local)) directly via the ScalarE
    activation accumulate port; the host adds the 8 partial sums and
    assembles the cross-entropy with the exact fp32 target logit
    (x_norm[b] . f[y_b] / TEMP, gathered on host).
  - EMA scatter update: only the <=256 targeted rows per bank change.  The
    per-sample updates are sharded across cores (32 samples/core, 3 banks):
    each core computes r = m*f[y] + (1-m)*x_norm and r/||r|| in fp32 on
    device (DVE + ACT sqrt + Newton-refined rsqrt).  The host scatters the
    updated rows into a copy of each bank; duplicate targets (in-order
    chained semantics of the reference scan) are chained on host.

Self-contained: hardcodes all shapes; no imports from the problem directory.
"""

import numpy as np
import ml_dtypes

# Problem constants (fixed by the task)
NUM_FEATURES = 2048
NUM_SAMPLES = 32768
BATCH = 256
TEMP = 0.05
MOMENTUM = 0.2
LAMBDA2 = 0.5
MU = 1.0

N_CORES = 8
SHARD = NUM_SAMPLES // N_CORES      # 4096 bank rows per core
NQ = 4                              # s-quarters per shard
QS = SHARD // NQ                    # 1024 columns per quarter
KC = NUM_FEATURES // 128            # 16 contraction chunks of 128
SPC = BATCH // N_CORES              # 32 EMA samples per core
P_EMA = 3 * SPC                     # 96 EMA rows per core (3 banks x 32)

BF16 = ml_dtypes.bfloat16

_BUILD_CACHE = {}


def _build_bass(m):
    """Build the per-core Bass program (same NEFF for all 8 cores)."""
    import concourse.bacc as bacc
    import concourse.mybir as mybir
    import concourse.tile as tile
    from contextlib import ExitStack

    fp32 = mybir.dt.float32
    bf16 = mybir.dt.bfloat16
    Alu = mybir.AluOpType
    Act = mybir.ActivationFunctionType
    K = NUM_FEATURES

    # Bacc (not plain Bass): its compile() pass legalizes multi-sync-wait
    # instructions (generate_event_semaphores) — walrus codegen rejects >1
    # wait on most ISA structs.
    nc = bacc.Bacc(
        "TRN2",
        target_bir_lowering=False,
        debug=False,
        num_devices=N_CORES,
        enable_asserts=False,
    )

    ft = nc.dram_tensor("ft", [3, NQ, K, QS], bf16, kind="ExternalInput")
    xt = nc.dram_tensor("xt", [3, K, BATCH], bf16, kind="ExternalInput")
    g = nc.dram_tensor("g", [P_EMA, K], fp32, kind="ExternalInput")
    xs = nc.dram_tensor("xs", [P_EMA, K], fp32, kind="ExternalInput")
    se = nc.dram_tensor("se", [128, 6], fp32, kind="ExternalOutput")
    eo = nc.dram_tensor("eo", [P_EMA, K], fp32, kind="ExternalOutput")

    fm = float(m)
    fm1 = float(1.0 - m)

    with ExitStack() as ctx:
        tc = ctx.enter_context(tile.TileContext(nc))
        singles = ctx.enter_context(tc.tile_pool(name="singles", bufs=1))
        ftp = ctx.enter_context(tc.tile_pool(name="ftp", bufs=2))
        psp = ctx.enter_context(tc.tile_pool(name="psp", bufs=8, space="PSUM"))

        # Resident stationary operand: (x_norm/TEMP).T, bf16, all 3 banks.
        xt_sb = singles.tile([128, 3, KC, BATCH], bf16)
        for b in range(3):
            nc.sync.dma_start(
                out=xt_sb[:, b], in_=xt[b].rearrange("(kc p) m -> p kc m", p=128)
            )

        # ---- EMA row build (early: runs on DVE while PE streams matmuls) ----
        gt = singles.tile([P_EMA, K], fp32)
        nc.sync.dma_start(out=gt, in_=g.ap())
        xst = singles.tile([P_EMA, K], fp32)
        nc.sync.dma_start(out=xst, in_=xs.ap())
        xpre = singles.tile([P_EMA, K], fp32)
        nc.vector.tensor_scalar_mul(xpre, xst, fm1)
        gpre = singles.tile([P_EMA, K], fp32)
        nc.vector.tensor_scalar_mul(gpre, gt, fm)
        r = singles.tile([P_EMA, K], fp32)
        nc.vector.tensor_add(r, gpre, xpre)
        rsq = singles.tile([P_EMA, K], fp32)
        nc.vector.tensor_mul(rsq, r, r)
        sq = singles.tile([P_EMA, 1], fp32)
        nc.vector.reduce_sum(out=sq, in_=rsq, axis=mybir.AxisListType.X)

        # ---- main: 3 banks x 4 quarters; 8 PSUM banks rotate ----
        acc = singles.tile([128, 48], fp32)  # sumexp partials: (bank*2+mi)*8 + q*2+nb
        for b in range(3):
            for q in range(NQ):
                ftt = ftp.tile([128, KC, QS], bf16, tag="ftt")
                nc.sync.dma_start(
                    out=ftt, in_=ft[b, q].rearrange("(kc p) s -> p kc s", p=128)
                )
                for mi in range(2):
                    for nb in range(QS // 512):
                        ps = psp.tile([128, 512], fp32, tag="ps")
                        for kc in range(KC):
                            nc.tensor.matmul(
                                ps,
                                xt_sb[:, b, kc, mi * 128:(mi + 1) * 128],
                                ftt[:, kc, nb * 512:(nb + 1) * 512],
                                start=(kc == 0),
                                stop=(kc == KC - 1),
                            )
                        idx = (b * 2 + mi) * 8 + q * 2 + nb
                        # exp in place on the PSUM tile: keeps this activation
                        # at a single sync-wait (PE sem) — the S3D3 accum
                        # struct has only one wait slot.
                        nc.scalar.activation(
                            out=ps, in_=ps, func=Act.Exp,
                            accum_out=acc[:, idx:idx + 1],
                        )

        # ---- reduce sumexp partials and store ----
        se_sb = singles.tile([128, 6], fp32)
        nc.vector.reduce_sum(
            out=se_sb,
            in_=acc.rearrange("p (g q) -> p g q", g=6),
            axis=mybir.AxisListType.X,
        )
        nc.sync.dma_start(out=se.ap(), in_=se_sb)

        # ---- EMA tail: r / ||r||, with Newton-refined 1/sqrt ----
        sqt = singles.tile([P_EMA, 1], fp32)
        nc.scalar.activation(out=sqt, in_=sq, func=Act.Sqrt)
        inv = singles.tile([P_EMA, 1], fp32)
        nc.vector.reciprocal(inv, sqt)
        t2 = singles.tile([P_EMA, 1], fp32)
        for _ in range(2):
            nc.vector.tensor_mul(t2, inv, inv)
            nc.vector.tensor_mul(t2, t2, sq)
            nc.vector.tensor_scalar(
                out=t2, in0=t2, scalar1=-0.5, scalar2=1.5,
                op0=Alu.mult, op1=Alu.add,
            )
            nc.vector.tensor_mul(inv, inv, t2)
        ro = singles.tile([P_EMA, K], fp32)
        nc.vector.tensor_scalar_mul(ro, r, inv)
        nc.sync.dma_start(out=eo.ap(), in_=ro)

    nc.finalize()
    return nc


def get_bass(m):
    key = round(float(m), 6)
    if key not in _BUILD_CACHE:
        _BUILD_CACHE[key] = _build_bass(m)
    return _BUILD_CACHE[key]


def _normalize_rows(a):
    a = np.asarray(a, dtype=np.float32)
    return a / np.linalg.norm(a, axis=1, keepdims=True).astype(np.float32)


def prepare(inputs, inputs_up, inputs_down, inputs_teacher, inputs_up_teacher,
            inputs_down_teacher, targets, epoch, features, features_up,
            features_down):
    """Host-side prep: normalize, shard, transpose/cast.  Returns everything
    needed to run the device program and assemble outputs."""
    ep = int(np.asarray(epoch))
    m = 1.0 if ep == -1 else MOMENTUM
    t = np.asarray(targets).astype(np.int64)

    xns = [_normalize_rows(a) for a in (inputs, inputs_up, inputs_down)]
    tns = [_normalize_rows(a) for a in
           (inputs_teacher, inputs_up_teacher, inputs_down_teacher)]
    banks = [np.asarray(a, dtype=np.float32)
             for a in (features, features_up, features_down)]

    # Stationary operand: (x_norm / TEMP).T in bf16, [3, K, B]
    xt_np = np.stack([
        np.ascontiguousarray((xn / TEMP).T).astype(BF16) for xn in xns
    ])

    # Bank shards, transposed to [NQ, K, QS] blocks per core, bf16.
    ft_cores = []
    for c in range(N_CORES):
        per_bank = []
        for fb in banks:
            shard = fb[c * SHARD:(c + 1) * SHARD]          # [4096, K] f32
            blk = shard.reshape(NQ, QS, NUM_FEATURES)      # [NQ, QS, K]
            per_bank.append(blk.transpose(0, 2, 1).astype(BF16))  # [NQ, K, QS]
        ft_cores.append(np.ascontiguousarray(np.stack(per_bank)))

    # EMA inputs: gathered bank rows + x_norm rows, sharded by sample index.
    gathered = [fb[t] for fb in banks]                      # [3][256, K] f32
    g_cores, xs_cores = [], []
    for c in range(N_CORES):
        sl = slice(c * SPC, (c + 1) * SPC)
        g_cores.append(np.ascontiguousarray(
            np.concatenate([ga[sl] for ga in gathered])))   # [96, K]
        xs_cores.append(np.ascontiguousarray(
            np.concatenate([xn[sl] for xn in xns])))        # [96, K]

    in_maps = [
        {"ft": ft_cores[c], "xt": xt_np, "g": g_cores[c], "xs": xs_cores[c]}
        for c in range(N_CORES)
    ]
    ctx = {
        "m": m, "targets": t, "xns": xns, "tns": tns, "banks": banks,
        "gathered": gathered,
    }
    return in_maps, ctx


def assemble(results, ctx):
    """Combine per-core outputs into (loss, nf, nfu, nfd)."""
    m = ctx["m"]
    t = ctx["targets"]
    xns, tns, banks = ctx["xns"], ctx["tns"], ctx["banks"]
    gathered = ctx["gathered"]

    # Global softmax denominators: sum the 8 local sum-exp shards.
    S = np.zeros((3, BATCH), dtype=np.float64)
    for res in results:
        se = np.asarray(res["se"], dtype=np.float64)        # [128, 6]
        for b in range(3):
            for mi in range(2):
                S[b, mi * 128:(mi + 1) * 128] += se[:, b * 2 + mi]

    # Exact target logits (fp64 on host; part of loss assembly).
    ces = []
    for b in range(3):
        l_tgt = np.einsum(
            "ij,ij->i", xns[b].astype(np.float64),
            gathered[b].astype(np.float64)) / TEMP
        ces.append(float(np.mean(-l_tgt + np.log(S[b]))))

    lds = [float(((xns[b].astype(np.float64) - tns[b].astype(np.float64)) ** 2)
                 .mean(0).sum()) for b in range(3)]

    loss = ((1.0 - LAMBDA2) * (ces[0] + MU * lds[0])
            + LAMBDA2 * (ces[1] + MU * lds[1])
            + LAMBDA2 * (ces[2] + MU * lds[2]))

    # EMA rows per bank, re-assembled in sample order.
    ema_rows = np.empty((3, BATCH, NUM_FEATURES), dtype=np.float32)
    for c, res in enumerate(results):
        eo = np.asarray(res["eo"], dtype=np.float32)        # [96, K]
        for b in range(3):
            ema_rows[b, c * SPC:(c + 1) * SPC] = eo[b * SPC:(b + 1) * SPC]

    outs = []
    fm = np.float32(m)
    fm1 = np.float32(1.0 - m)
    for b in range(3):
        out = banks[b].copy()
        seen = set()
        for i in range(BATCH):
            y = int(t[i])
            if y in seen:
                # chained duplicate: reference applies updates sequentially
                row = fm * out[y] + fm1 * xns[b][i]
                row = row / np.float32(np.linalg.norm(row))
            else:
                row = ema_rows[b, i]
                seen.add(y)
            out[y] = row
        outs.append(out)

    return np.float32(loss), outs[0], outs[1], outs[2]


def kernel(**inputs):
    in_maps, ctx = prepare(**inputs)
    nc = get_bass(ctx["m"])
    from concourse.bass_utils import run_bass_kernel_spmd
    res = run_bass_kernel_spmd(nc, in_maps, core_ids=list(range(N_CORES)))
    return assemble(res.results, ctx)
